# revision 1
# baseline (speedup 1.0000x reference)
# Trainium2 Bass kernel for nn_AttentionCombiner (self-attention where Q=K=V
# come from concat(output1, output2), followed by an output projection).
#
# Sharding: 8 cores = 4 batches x 2 q-halves. Each core computes all 8 heads
# for 1024 query rows of one batch, plus the full fc_out contraction for those
# rows (all features local). Only the softmax row-sums need communication:
# with the symmetric-energy trick each core's exp pass produces half of every
# row-sum via ACT accum_out; a pairwise 8KB ReduceScatter per head completes
# them.
#
# Key algebra (per batch, per head; X = combined features [2048, 128]):
#   E = X X^T (symmetric), S = exp(E/sqrt(d))
#   out^T[d, q] = sum_l X[l, d] S[l, q]          (unnormalized)
#   final[q, o] = sum_h (1/r_h[q]) sum_d out^T_h[d, q] W[h*128+d, o] + b[o]
#   r_h[q] = sum_l S_h[l, q]   (computed as column-sums of S tiles, which by
#                               symmetry equal the row-sums)
# This layout needs zero on-chip transposes: S tiles [l_part, q_free] are
# produced by MM1 and consumed directly as MM2's moving operand, and the
# softmax normalization is applied as a per-partition scalar at the fc
# combine, where q sits on partitions.

import numpy as np
import ml_dtypes

N, S, D_IN, HEADS = 4, 2048, 512, 8
HEAD_DIM = 128          # 2*D_IN // HEADS
DF = 2 * D_IN           # 1024 combined features
QH = S // 2             # 1024 rows per core
NB = S // 128           # 16 l-blocks
QB = QH // 128          # 8 q-blocks per core
ISQ = 1.0 / float(np.sqrt(np.float32(HEAD_DIM)))

_CACHED_NC = None


def _build_nc(no_collective=False):
    import concourse.mybir as mybir
    import concourse.tile as tile
    from concourse import bacc
    from concourse.bass import ts

    f32 = mybir.dt.float32
    bf16 = mybir.dt.bfloat16
    Exp = mybir.ActivationFunctionType.Exp
    mult = mybir.AluOpType.mult
    add = mybir.AluOpType.add

    nc = bacc.Bacc("TRN2", target_bir_lowering=False, debug=False, num_devices=8)

    xt = nc.dram_tensor("xt", [DF, S], bf16, kind="ExternalInput")      # X^T
    xtq = nc.dram_tensor("xtq", [DF, QH], bf16, kind="ExternalInput")   # X^T my-half cols
    x = nc.dram_tensor("x", [S, DF], bf16, kind="ExternalInput")        # X
    w = nc.dram_tensor("w", [DF, D_IN], bf16, kind="ExternalInput")     # W_out
    bias = nc.dram_tensor("bias", [128, D_IN], f32, kind="ExternalInput")
    out = nc.dram_tensor("out", [QH, D_IN], f32, kind="ExternalOutput")

    RG = [[0, 1], [2, 3], [4, 5], [6, 7]]

    with tile.TileContext(nc) as tc:
        with (
            tc.tile_pool(name="persist", bufs=1) as pers,
            tc.tile_pool(name="spool", bufs=4) as spool,
            tc.tile_pool(name="outp", bufs=3) as outp,
            tc.tile_pool(name="rpool", bufs=3) as rpool,
            tc.tile_pool(name="psE", bufs=2, space="PSUM") as psE,
            tc.tile_pool(name="psO", bufs=1, space="PSUM") as psO,
            tc.tile_pool(name="psFC", bufs=2, space="PSUM") as psFC,
            tc.tile_pool(name="dram", bufs=1, space="DRAM") as dram,
        ):
            # ---- persistent SBUF data ----
            xt_sb = pers.tile([128, HEADS, S], bf16, name="xt_sb")
            xtq_sb = pers.tile([128, HEADS, QH], bf16, name="xtq_sb")
            x_sb = pers.tile([128, NB, DF], bf16, name="x_sb")
            w_sb = pers.tile([128, HEADS, D_IN], bf16, name="w_sb")
            bias_sb = pers.tile([128, D_IN], f32, name="bias_sb")

            xt_r = xt.ap().rearrange("(h p) s -> p h s", p=128)
            xtq_r = xtq.ap().rearrange("(h p) s -> p h s", p=128)
            x_r = x.ap().rearrange("(o p) f -> p o f", p=128)
            w_r = w.ap().rearrange("(h p) o -> p h o", p=128)

            # Front-load head 0 / early blocks so compute can start promptly.
            nc.sync.dma_start(xt_sb[:, 0, :], xt_r[:, 0, :])
            nc.sync.dma_start(xtq_sb[:, 0, :], xtq_r[:, 0, :])
            for i in range(NB):
                nc.sync.dma_start(x_sb[:, i, :], x_r[:, i, :])
            for h in range(1, HEADS):
                nc.sync.dma_start(xt_sb[:, h, :], xt_r[:, h, :])
                nc.sync.dma_start(xtq_sb[:, h, :], xtq_r[:, h, :])
            for h in range(HEADS):
                nc.sync.dma_start(w_sb[:, h, :], w_r[:, h, :])
            nc.sync.dma_start(bias_sb[:], bias.ap())

            # fc accumulators, persist across heads
            accs = []
            for j in range(QB):
                a = pers.tile([128, D_IN], f32, name=f"acc{j}")
                accs.append(a)

            # deferred fc work for the previous head, spread through the next
            # head's i-loop so PE/DVE stalls on the collective never block
            # the attention stream
            pending_fc = []

            def emit_fc_step():
                if pending_fc:
                    pending_fc.pop(0)()

            def emit_fc_head(h, outT, recip):
                for j in range(QB):
                    def step(h=h, j=j, outT=outT, recip=recip):
                        pfc = psFC.tile([128, D_IN], f32, tag="pfc", name="pfc")
                        nc.tensor.matmul(pfc[:], outT[:, ts(j, 128)],
                                         w_sb[:, h, :], start=True, stop=True)
                        if h == 0:
                            nc.vector.scalar_tensor_tensor(
                                accs[j][:], pfc[:], recip[:, j : j + 1],
                                bias_sb[:], mult, add)
                        else:
                            nc.vector.scalar_tensor_tensor(
                                accs[j][:], pfc[:], recip[:, j : j + 1],
                                accs[j][:], mult, add)
                    pending_fc.append(step)

            for h in range(HEADS):
                racc = pers.tile([128, NB], f32, name=f"racc{h}")
                bin_h = dram.tile([NB, 128], f32, name=f"bin{h}")
                bout_h = dram.tile([QB, 128], f32, name=f"bout{h}")

                pso = psO.tile([128, QH], f32, tag="pso", name="pso")
                pse_tiles = {}

                def mm1(i, h=h, pse_tiles=pse_tiles):
                    pse = psE.tile([128, QH], f32, tag="pse", name="pse")
                    lhs1 = xt_sb[:, h, ts(i, 128)]
                    nc.tensor.matmul(pse[:, 0:512], lhs1, xtq_sb[:, h, 0:512],
                                     start=True, stop=True)
                    nc.tensor.matmul(pse[:, 512:1024], lhs1,
                                     xtq_sb[:, h, 512:1024],
                                     start=True, stop=True)
                    pse_tiles[i] = pse

                mm1(0)
                mm1(1)
                for i in range(NB):
                    pse = pse_tiles.pop(i)
                    s_i = spool.tile([128, QH], bf16, tag="s", name="s_i")
                    nc.scalar.activation(s_i[:], pse[:], Exp, bias=0.0, scale=ISQ,
                                         accum_out=racc[:, i : i + 1])
                    if i + 2 < NB:
                        mm1(i + 2)
                    lhs2 = x_sb[:, i, ts(h, 128)]
                    nc.tensor.matmul(pso[:, 0:512], lhs2, s_i[:, 0:512],
                                     start=(i == 0), stop=(i == NB - 1))
                    nc.tensor.matmul(pso[:, 512:1024], lhs2, s_i[:, 512:1024],
                                     start=(i == 0), stop=(i == NB - 1))
                    # spread the previous head's fc work through this loop
                    emit_fc_step()

                with nc.allow_non_contiguous_dma(reason="8KB rowsum bounce"):
                    nc.sync.dma_start(
                        bin_h[:, :].rearrange("i p -> p i"), racc[:])
                if no_collective:
                    # timing-model variant: local stand-in for the pairwise RS
                    nc.sync.dma_start(bout_h[:, :], bin_h[0:QB, :])
                else:
                    nc.gpsimd.collective_compute(
                        "ReduceScatter",
                        add,
                        replica_groups=RG,
                        ins=[bin_h.opt()],
                        outs=[bout_h.opt()],
                    )
                rsum = rpool.tile([128, QB], f32, tag="rsum", name="rsum")
                with nc.allow_non_contiguous_dma(reason="4KB rowsum load"):
                    nc.sync.dma_start(
                        rsum[:], bout_h[:, :].rearrange("j p -> p j"))
                recip = rpool.tile([128, QB], f32, tag="recip", name="recip")
                nc.vector.reciprocal(recip[:], rsum[:])

                outT = outp.tile([128, QH], bf16, tag="outT", name="outT")
                nc.vector.tensor_copy(outT[:], pso[:])

                emit_fc_head(h, outT, recip)

            while pending_fc:
                emit_fc_step()

            for j in range(QB):
                nc.sync.dma_start(out.ap()[ts(j, 128), :], accs[j][:])

    nc.compile()
    return nc


def _get_nc():
    global _CACHED_NC
    if _CACHED_NC is None:
        _CACHED_NC = _build_nc()
    return _CACHED_NC


def kernel(output1, output2, W_out, b_out):
    from concourse.bass_utils import run_bass_kernel_spmd

    bf = ml_dtypes.bfloat16
    X = np.concatenate([np.asarray(output1), np.asarray(output2)], axis=2)  # [N,S,DF] f32
    Xb = X.astype(bf)
    Wb = np.ascontiguousarray(np.asarray(W_out).astype(bf))
    bias_full = np.ascontiguousarray(
        np.broadcast_to(np.asarray(b_out).astype(np.float32), (128, D_IN)))

    in_maps = []
    for c in range(8):
        n, half = c // 2, c % 2
        Xn = np.ascontiguousarray(Xb[n])            # [S, DF]
        XTn = np.ascontiguousarray(Xn.T)            # [DF, S]
        in_maps.append({
            "x": Xn,
            "xt": XTn,
            "xtq": np.ascontiguousarray(XTn[:, half * QH : (half + 1) * QH]),
            "w": Wb,
            "bias": bias_full,
        })

    nc = _get_nc()
    res = run_bass_kernel_spmd(nc, in_maps, core_ids=list(range(8)))

    full = np.empty((N, S, D_IN), np.float32)
    for c in range(8):
        n, half = c // 2, c % 2
        full[n, half * QH : (half + 1) * QH, :] = res.results[c]["out"]
    return full

